# revision 26
# baseline (speedup 1.0000x reference)
"""Trainium2 Bass kernel for nn_CNNStateEncoder (dense_cnn).

Network per row (B*S rows, 8 features each):
  conv1 2x2 on [1,2,4] -> 32ch x [1,3]   == h1[96]  = A1[96,8]  @ x[8],  relu(+b1)
  conv2 1x2 on [32,1,3] -> 32ch x [1,2]  == h2[64]  = A2[64,96] @ h1,    relu(+b2)
  linear 64->64                          == out[64] = Wp[64,64] @ h2 + bp

Mapping on each NeuronCore (data parallel over 8 cores, 65536 rows/core,
2048-row tiles; PE HAM stays at 1.2GHz on this part, so minimize matmul
count and maximize row/col-group concurrency):
  - rows live in the matmul free dim (feature-major chain)
  - input: DVE cast f32->bf16, gpsimd x4-replicate into 32-blocks, DVE
    StreamTranspose; row-chunk q's 8 features land at partitions 32q..32q+8
  - conv1: 4 concurrently-packed K=8 matmuls (row groups), one psum bank
    each (concurrent drains must hit distinct banks)
  - relu1: ONE contiguous ACT op over the 4 banks
  - conv2: K=96, N=512 matmuls; the tile's two 1024-row halves go to output
    col groups 0/64 and run concurrently (packed by partition halves)
  - linear: lhsT = activations (M=rows) -> row-major PSUM; 16 chunks issued
    as concurrent (row-group 0-1 x bank0, row-group 2-3 x bank1) pairs
  - out: single DVE bias-add+copy, single 512KB store
"""

import numpy as np
import ml_dtypes

B, S, FEAT, OUT = 64, 8192, 8, 64
NCORES = 8
ROWS_TOTAL = B * S
ROWS_CORE = ROWS_TOTAL // NCORES  # 65536
TILE_ROWS = 2048

BF16 = ml_dtypes.bfloat16

# ---------------------------------------------------------------------------
# numpy-side weight packing
# ---------------------------------------------------------------------------

def pack_weights(W1, b1, W2, b2, Wp, bp):
    W1 = np.asarray(W1, np.float32)
    W2 = np.asarray(W2, np.float32)
    Wp = np.asarray(Wp, np.float32)
    b1 = np.asarray(b1, np.float32)
    b2 = np.asarray(b2, np.float32)
    bp = np.asarray(bp, np.float32)

    # A1 [96, 8]: h1[o*3+j] = sum_{kh,kw} x[kh*4 + j + kw] * W1[o,0,kh,kw]
    A1 = np.zeros((96, 8), np.float32)
    for o in range(32):
        for j in range(3):
            for kh in range(2):
                for kw in range(2):
                    A1[o * 3 + j, kh * 4 + j + kw] += W1[o, 0, kh, kw]
    b1_96 = np.repeat(b1, 3).astype(np.float32)

    # A2 [64, 96]: h2[c*2+w] = sum_{i,kw} h1[i*3 + w + kw] * W2[c,i,0,kw]
    A2 = np.zeros((64, 96), np.float32)
    for c in range(32):
        for w in range(2):
            for i in range(32):
                for kw in range(2):
                    A2[c * 2 + w, i * 3 + w + kw] += W2[c, i, 0, kw]
    b2_64 = np.repeat(b2, 2).astype(np.float32)

    a1t = np.zeros((128, 96), np.float32)
    for q in range(4):
        a1t[32 * q:32 * q + 8, :] = A1.T
    a2t = np.zeros((96, 128), np.float32)
    a2t[:, 0:64] = A2.T
    a2t[:, 64:128] = A2.T
    wpt = np.zeros((128, 64), np.float32)
    wpt[0:64, :] = Wp.T
    wpt[64:128, :] = Wp.T
    b1c = b1_96.reshape(96, 1)
    b2c = np.concatenate([b2_64, b2_64]).reshape(128, 1)
    bpb = np.tile(bp, (128, TILE_ROWS // 128))  # [128, 1024]

    return {
        "a1t": a1t.astype(BF16),
        "a2t": a2t.astype(BF16),
        "wpt": wpt.astype(BF16),
        "b1c": b1c,
        "b2c": b2c,
        "bpb": bpb.astype(np.float32),
    }


# ---------------------------------------------------------------------------
# bass module
# ---------------------------------------------------------------------------

def build_nc(rows=ROWS_CORE):
    import concourse.bass as bass
    import concourse.bacc as bacc
    import concourse.mybir as mybir
    import concourse.tile as tile

    f32 = mybir.dt.float32
    bf16 = mybir.dt.bfloat16
    Relu = mybir.ActivationFunctionType.Relu
    Alu = mybir.AluOpType

    assert rows % TILE_ROWS == 0
    ntiles = rows // TILE_ROWS

    nc = bacc.Bacc(None, target_bir_lowering=False)

    x_d = nc.dram_tensor("x", [rows, FEAT], f32, kind="ExternalInput")
    a1t_d = nc.dram_tensor("a1t", [128, 96], bf16, kind="ExternalInput")
    a2t_d = nc.dram_tensor("a2t", [96, 128], bf16, kind="ExternalInput")
    wpt_d = nc.dram_tensor("wpt", [128, 64], bf16, kind="ExternalInput")
    b1c_d = nc.dram_tensor("b1c", [96, 1], f32, kind="ExternalInput")
    b2c_d = nc.dram_tensor("b2c", [128, 1], f32, kind="ExternalInput")
    bpb_d = nc.dram_tensor("bpb", [128, 1024], f32, kind="ExternalInput")
    out_d = nc.dram_tensor("out", [rows, OUT], f32, kind="ExternalOutput")

    with tile.TileContext(nc) as tc:
        with (
            tc.tile_pool(name="consts", bufs=1) as cpool,
            tc.tile_pool(name="xin", bufs=4) as xpool,
            tc.tile_pool(name="xbf", bufs=4) as xbpool,
            tc.tile_pool(name="xpad", bufs=4) as xppool,
            tc.tile_pool(name="xt", bufs=4) as xtpool,
            tc.tile_pool(name="h1s", bufs=3) as h1pool,
            tc.tile_pool(name="h2s", bufs=3) as h2pool,
            tc.tile_pool(name="osb", bufs=3) as opool,
            tc.tile_pool(name="ps_h1", bufs=1, space="PSUM") as ps_h1,
            tc.tile_pool(name="ps_h2", bufs=1, space="PSUM") as ps_h2,
            tc.tile_pool(name="ps_o", bufs=1, space="PSUM") as ps_o,
        ):
            a1t = cpool.tile([128, 96], bf16)
            a2t = cpool.tile([96, 128], bf16)
            wpt = cpool.tile([128, 64], bf16)
            b1c = cpool.tile([96, 1], f32)
            b2c = cpool.tile([128, 1], f32)
            bpb = cpool.tile([128, 1024], f32)
            nc.sync.dma_start(a1t[:], a1t_d[:])
            nc.sync.dma_start(a2t[:], a2t_d[:])
            nc.sync.dma_start(wpt[:], wpt_d[:])
            nc.sync.dma_start(b1c[:], b1c_d[:])
            nc.sync.dma_start(b2c[:], b2c_d[:])
            nc.sync.dma_start(bpb[:], bpb_d[:])

            for t in range(ntiles):
                n0 = t * TILE_ROWS
                # ---- load + cast + replicate + transpose ----
                x_sb = xpool.tile([128, 128], f32)
                nc.sync.dma_start(
                    x_sb[:],
                    x_d[n0:n0 + TILE_ROWS, :].rearrange("(p r) f -> p (r f)", p=128),
                )
                x_bf = xbpool.tile([128, 128], bf16)
                nc.vector.tensor_copy(x_bf[:], x_sb[:])
                # x_pad[p, 32a+8g+f] = x_bf[p, 8a+f] = x[n0 + 16p + a, f]
                x_pad = xppool.tile([128, 512], bf16)
                rep_ap = (
                    x_bf[:]
                    .rearrange("p (a f) -> p a f", f=8)
                    .unsqueeze(2)
                    .broadcast_to((128, 16, 4, 8))
                )
                nc.gpsimd.tensor_copy(x_pad[:], rep_ap)
                # xt[32q+8g+f, 32a+v] = x[n0 + 512q + 16v + a, f]
                xt = xtpool.tile([128, 512], bf16)
                nc.vector.transpose(xt[:], x_pad[:])

                # ---- conv1: 4 packed K=8 matmuls, one psum bank each ----
                # rhs streams (v outer, a inner) so bank q's col j = row
                # n0 + 512q + j
                h1ps = ps_h1.tile([96, 2048], f32)
                for q in range(4):
                    rhs = xt[32 * q:32 * q + 8, :].rearrange("k (a v) -> k v a", v=32)
                    nc.tensor.matmul(
                        h1ps[:, 512 * q:512 * q + 512],
                        a1t[32 * q:32 * q + 8, :],
                        rhs,
                        tile_position=(32 * q, 0),
                    )
                # ---- relu1 (+b1): ONE contiguous ACT op ----
                h1s = h1pool.tile([96, 2048], bf16)
                nc.scalar.activation(h1s[:], h1ps[:], Relu, bias=b1c[:])

                # ---- conv2: 4 matmuls; the two 1024-row halves of the tile
                # land on col groups 0/64 and run concurrently ----
                h2ps_a = ps_h2.tile([128, 512], f32)
                h2ps_b = ps_h2.tile([128, 512], f32)
                for ps, lo in ((h2ps_a, 0), (h2ps_b, 512)):
                    for h in (0, 1):
                        nc.tensor.matmul(
                            ps[64 * h:64 * h + 64, :],
                            a2t[:, 64 * h:64 * h + 64],
                            h1s[:, 1024 * h + lo:1024 * h + lo + 512],
                            tile_position=(0, 64 * h),
                        )
                # ---- relu2 (+b2): bank A on ACT, bank B on DVE ----
                h2s_a = h2pool.tile([128, 512], bf16)
                h2s_b = h2pool.tile([128, 512], bf16)
                nc.scalar.activation(h2s_a[:], h2ps_a[:], Relu, bias=b2c[:])
                nc.vector.tensor_scalar(
                    h2s_b[:], h2ps_b[:], b2c[:], 0.0, Alu.add, Alu.max
                )

                # ---- linear: 16 chunks of 128 rows; issue (h=0, h=1) chunk
                # pairs adjacently -> concurrent row groups + distinct banks.
                # chunk c covers rows [n0+128c, +128); h = c//8 selects the
                # h2 partition half, bank = c//8 too (cols 64c).
                outps = ps_o.tile([128, 1024], f32)
                for cc in range(8):
                    for h in (0, 1):
                        c = 8 * h + cc
                        X = (c // 4) % 2
                        h2s = h2s_a if X == 0 else h2s_b
                        col = 128 * (c % 4)
                        nc.tensor.matmul(
                            outps[:, 64 * c:64 * c + 64],
                            h2s[64 * h:64 * h + 64, col:col + 128],
                            wpt[64 * h:64 * h + 64, :],
                            start=(cc == 0),
                            stop=(cc == 7),
                            tile_position=(64 * h, 0),
                        )
                # ---- bias + store ----
                out_sb = opool.tile([128, 1024], f32)
                nc.vector.tensor_tensor(out_sb[:], outps[:], bpb[:], Alu.add)
                nc.sync.dma_start(
                    out_d[n0:n0 + TILE_ROWS, :].rearrange("(c p) j -> p c j", p=128),
                    out_sb[:],
                )

    nc.compile()
    return nc


# ---------------------------------------------------------------------------
# entry point
# ---------------------------------------------------------------------------

_CACHE = {}


def _get_nc(rows=ROWS_CORE):
    if rows not in _CACHE:
        _CACHE[rows] = build_nc(rows)
    return _CACHE[rows]


def kernel(x, W1, b1, W2, b2, Wp, bp):
    from concourse.bass_utils import run_bass_kernel_spmd

    x = np.ascontiguousarray(np.asarray(x, np.float32)).reshape(ROWS_TOTAL, FEAT)
    consts = pack_weights(W1, b1, W2, b2, Wp, bp)

    nc = _get_nc()
    in_maps = []
    for c in range(NCORES):
        m = dict(consts)
        m["x"] = x[c * ROWS_CORE:(c + 1) * ROWS_CORE]
        in_maps.append(m)

    res = run_bass_kernel_spmd(nc, in_maps, core_ids=list(range(NCORES)))
    out = np.concatenate([r["out"] for r in res.results], axis=0)
    return out.reshape(B, S, OUT)
